# revision 23
# baseline (speedup 1.0000x reference)
"""GAT multi-head attention (nn_GATMHAEfficient) on 8 Trainium2 NeuronCores.

Data-parallel over batch B=32 -> 4 graphs per core. Host folds W/Wal/War
into wcat (128 x 152): per-head [W_h | 0] blocks (the 0 column becomes the
on-chip "ones" column so the aggregation matmul also produces the softmax
denominator), then W@Wal (a_i) and W@War (a_j) columns.

Score pipeline: exp(leaky_relu(a_i + a_j)) == max(E_i*E_j, F_i*F_j) with
E = exp(a), F = exp(0.2*a) (exact identity: for s>0 exp(s) wins, for s<0
exp(0.2s) wins). The exps move to the small per-node vectors, so the N^2
work is cheap bf16 ALU ops instead of ACT table ops:
  u  = (Ei_bc * Ej_ptr)            DVE tensor_scalar (bf16 4x mode)
  w  = (Fi_bc * Fj_ptr) max u      Pool STT (fused) or DVE TS+TT
  P  = w * notm                    DVE tensor_tensor (bf16 2x mode)
A tunable fraction of tiles instead uses the direct ACT pipeline
(Prelu(ai_bc + aj_bias) in f32, Exp -> bf16) to load-balance ACT vs
DVE vs Pool. P (bf16) streams into the PE aggregation matmul at
1 cycle/row; per-head [g|1] stationary gives numerator + denominator.
"""

import json

import numpy as np

import concourse.bass as bass
import concourse.mybir as mybir
import concourse.tile as tile
from concourse.vector_clock import ScopedClock, VectorClock

F32 = mybir.dt.float32
BF16 = mybir.dt.bfloat16
AF = mybir.ActivationFunctionType
ALU = mybir.AluOpType

B, N, NI, H, D = 32, 1024, 128, 8, 16
NCORES = 8
B_SH = B // NCORES          # graphs per core
C = N // 128                # j-chunks of 128
NEG_SLOPE = 0.2
GEXT = H * (D + 1)          # 136: per-head [g(16) | ones]
WCOLS = GEXT + 2 * H        # 152: + a_i cols + a_j cols
GRP = 2                     # chunks per score tile
NG = C // GRP               # score tile groups per (b, h)

# flavor pattern over groups: OLD = ACT prelu+exp, NP = DVE-TS + Pool-STT,
# ND = all-DVE. Tuned so ACT/DVE/Pool busy times balance
# (target ~12 OLD / 19 NP / 1 ND per 32).
PATTERN = ["OLD", "PP", "PP", "OLD", "PP", "ND", "OLD", "PP",
           "PP", "OLD", "PP", "ND", "OLD", "PP", "PP", "PP",
           "PP", "ND", "OLD", "PP", "PP", "OLD", "PP", "ND",
           "OLD", "PP", "PP", "OLD", "PP", "ND", "PP", "OLD"]

# ---------------------------------------------------------------------------
# Workarounds for this container's walrus build: it accepts at most ONE
# sync-wait per instruction, but Tile's sem-assignment (and its final drain)
# attach several. Split the excess onto dedicated single-wait EventSemaphore
# carrier instructions in the serialized BIR.


def _legalize_sync_waits(d, max_waits=1):
    for fn in d["functions"]:
        for bb in fn["blocks"]:
            new_insts = []
            for inst in bb["instructions"]:
                si = inst.get("sync_info") or {}
                w = si.get("on_wait") or []
                if len(w) > max_waits:
                    for k, we in enumerate(w[:-max_waits]):
                        new_insts.append(
                            {
                                "debug": inst.get("debug", 0),
                                "engine": inst["engine"],
                                "ins": [],
                                "outs": [],
                                "name": f"{inst['name']}_xw{k}",
                                "opcode": "EventSemaphore",
                                "sync_info": {"on_update": [], "on_wait": [we]},
                            }
                        )
                    si["on_wait"] = w[-max_waits:]
                new_insts.append(inst)
            bb["instructions"] = new_insts


def _wrap_to_json(nc):
    raw = nc.to_json_bytes

    def patched():
        d = json.loads(raw())
        _legalize_sync_waits(d)
        return json.dumps(d).encode()

    nc.to_json_bytes = patched


def _split_drain_and_barrier(self, tick_clock, wait_clock):
    # One drain per logical processor so each carries a single sem wait.
    gc = tick_clock.global_clock
    n = len(gc)
    for proc in range(n):
        t = gc[proc]
        if t > 0:
            dr = self.nc.sync.drain()
            pc = VectorClock([t if i == proc else 0 for i in range(n)])
            wait_clock.add_sem_waits(dr.ins, ScopedClock({None: pc}))
    self.nc.all_engine_barrier()
    popped = self.nc._tile_sem_poison_stack.pop()
    assert popped is self._sem_poison
    self.nc.clear_and_free_semaphores(list(self.sems.allocated().values()))
    self.nc.all_engine_barrier()


tile.TileContext._drain_and_barrier = _split_drain_and_barrier

# ---------------------------------------------------------------------------


def build_nc():
    nc = bass.Bass()
    hT = nc.dram_tensor("hT", [B_SH, NI, N], F32, kind="ExternalInput")
    notmT = nc.dram_tensor("notmT", [B_SH, N, N], BF16, kind="ExternalInput")
    wcat = nc.dram_tensor("wcat", [NI, WCOLS], F32, kind="ExternalInput")
    id17 = nc.dram_tensor("id17", [D + 1, D + 1], F32, kind="ExternalInput")
    out = nc.dram_tensor("out", [B_SH, N, H * D], F32, kind="ExternalOutput")
    # per (b,h): rows [Ej | Ei | Fi | ai] staged for the broadcast DMA
    ef_scr = nc.dram_tensor("ef_scr", [B_SH, H, 4, N], BF16)

    from contextlib import ExitStack

    with ExitStack() as ctx:
        tc = ctx.enter_context(tile.TileContext(nc))
        const_p = ctx.enter_context(tc.tile_pool(name="const", bufs=1))
        hb_p = ctx.enter_context(tc.tile_pool(name="hb", bufs=2))
        nm_p = ctx.enter_context(tc.tile_pool(name="nm", bufs=2))
        gx_p = ctx.enter_context(tc.tile_pool(name="gx", bufs=2))
        aj_p = ctx.enter_context(tc.tile_pool(name="aj", bufs=2))
        ei_p = ctx.enter_context(tc.tile_pool(name="ei", bufs=2))
        bc_p = ctx.enter_context(tc.tile_pool(name="bc", bufs=4))
        ejr_p = ctx.enter_context(tc.tile_pool(name="ejr", bufs=4))
        scu_p = ctx.enter_context(tc.tile_pool(name="scu", bufs=6))
        scv_p = ctx.enter_context(tc.tile_pool(name="scv", bufs=2))
        scpm_p = ctx.enter_context(tc.tile_pool(name="scpm", bufs=3))
        sct2_p = ctx.enter_context(tc.tile_pool(name="sct2", bufs=3))
        ut_p = ctx.enter_context(tc.tile_pool(name="ut", bufs=2))
        rc_p = ctx.enter_context(tc.tile_pool(name="rc", bufs=2))
        ob_p = ctx.enter_context(tc.tile_pool(name="ob", bufs=2))
        xps_p = ctx.enter_context(tc.tile_pool(name="xps", bufs=2, space="PSUM"))
        ups_p = ctx.enter_context(tc.tile_pool(name="ups", bufs=2, space="PSUM"))
        up_p = ctx.enter_context(tc.tile_pool(name="up", bufs=2, space="PSUM"))
        if True:
            wcat_s = const_p.tile([NI, WCOLS], F32)
            nc.sync.dma_start(out=wcat_s[:], in_=wcat[:])
            id17_s = const_p.tile([D + 1, D + 1], F32)
            nc.sync.dma_start(out=id17_s[:], in_=id17[:])
            onec = const_p.tile([128, 1], BF16)
            nc.vector.memset(onec[:], 1.0)

            def _prep(b):
                hbT = hb_p.tile([NI, N], F32)
                nc.sync.dma_start(out=hbT[:], in_=hT[b])

                # a_i rows first (they gate the ef_scr round-trip and the
                # per-head broadcast DMAs), then the X-proj chunks.
                # Half-tiles keep each PSUM tile at one bank.
                # staging rows 0=Ej 1=Ei 2=Fi 3=ai
                eif8 = ei_p.tile([H, 4, N], BF16, tag="eif8")
                for half in range(2):
                    sl = slice(half * 512, (half + 1) * 512)
                    XT_ps = xps_p.tile([H, 512], F32, tag="xv")
                    nc.tensor.matmul(
                        XT_ps[:],
                        lhsT=wcat_s[:, GEXT : GEXT + H],
                        rhs=hbT[:, sl],
                        start=True,
                        stop=True,
                    )
                    nc.scalar.activation(out=eif8[:, 1, sl], in_=XT_ps[:], func=AF.Exp, scale=1.0)
                    nc.scalar.activation(out=eif8[:, 2, sl], in_=XT_ps[:], func=AF.Exp, scale=0.2)
                    nc.vector.tensor_copy(eif8[:, 3, sl], XT_ps[:])
                # a_j rows -> E_j rows (stationary side of the PE rank-1
                # outer-product flavor)
                for half in range(2):
                    sl = slice(half * 512, (half + 1) * 512)
                    AJ_ps = xps_p.tile([H, 512], F32, tag="xv")
                    nc.tensor.matmul(
                        AJ_ps[:],
                        lhsT=wcat_s[:, GEXT + H : WCOLS],
                        rhs=hbT[:, sl],
                        start=True,
                        stop=True,
                    )
                    nc.scalar.activation(out=eif8[:, 0, sl], in_=AJ_ps[:], func=AF.Exp, scale=1.0)
                for k in range(4):
                    dst = bass.AP(
                        tensor=ef_scr,
                        offset=(b * H * 4 + k) * N,
                        ap=[[4 * N, H], [1, N]],
                    )
                    nc.sync.dma_start(out=dst, in_=eif8[:, k, :])

                # X = h_b @ wcat chunk by chunk; g -> bf16, a_j cols -> f32
                gext = gx_p.tile([128, C, GEXT], BF16, tag="gx")
                aj_s = aj_p.tile([128, C, H], F32, tag="aj")
                ej_s = aj_p.tile([128, C, H], F32, tag="ej")
                fj_s = aj_p.tile([128, C, H], F32, tag="fj")
                for c in range(C):
                    X_ps = xps_p.tile([128, WCOLS], F32, tag="xv")
                    nc.tensor.matmul(
                        X_ps[:],
                        lhsT=hbT[:, c * 128 : (c + 1) * 128],
                        rhs=wcat_s[:],
                        start=True,
                        stop=True,
                    )
                    if c % 2 == 0:
                        nc.vector.tensor_copy(gext[:, c, :], X_ps[:, 0:GEXT])
                    else:
                        nc.scalar.copy(out=gext[:, c, :], in_=X_ps[:, 0:GEXT])
                    nc.vector.tensor_copy(aj_s[:, c, :], X_ps[:, GEXT + H : WCOLS])
                # per-head ones column (bf16 1.0 exact)
                ones_view = bass.AP(
                    tensor=gext.tensor,
                    offset=gext.offset + D,
                    ap=[gext.ap[0], [GEXT, C], [D + 1, H]],
                )
                ones_src = bass.AP(
                    tensor=onec.tensor,
                    offset=onec.offset,
                    ap=[onec.ap[0], [0, C], [0, H]],
                )
                nc.vector.tensor_copy(ones_view, ones_src)
                # E_j = exp(a_j), F_j = exp(0.2 a_j) per-partition scalars
                nc.scalar.activation(out=ej_s[:], in_=aj_s[:], func=AF.Exp, scale=1.0)
                nc.scalar.activation(out=fj_s[:], in_=aj_s[:], func=AF.Exp, scale=0.2)

                # notm last: it is a big transfer only needed by phase B, and
                # the serial DMA resource must first serve the ef_scr/bc3
                # chain that gates phase A
                notm = nm_p.tile([128, C, N], BF16)
                nc.sync.dma_start(
                    out=notm[:], in_=notmT[b].rearrange("(c p) i -> p c i", p=128)
                )

                return notm, gext, aj_s, ej_s, fj_s, eif8

            preps = {0: _prep(0)}
            bc3s = {}

            def issue_bc(bb, hh):
                t = bc_p.tile([128, 3, N], BF16, tag="bc3", name=f"bc3_{bb}_{hh}")
                nc.sync.dma_start(
                    out=t[:],
                    in_=bass.AP(
                        tensor=ef_scr,
                        offset=(bb * H + hh) * 4 * N + N,
                        ap=[[0, 128], [1, 3 * N]],
                    ),
                )
                ejr = ejr_p.tile([1, 2, N], BF16, tag="ejr", name=f"ejr_{bb}_{hh}")
                nc.sync.dma_start(
                    out=ejr[:],
                    in_=bass.AP(
                        tensor=ef_scr,
                        offset=(bb * H + hh) * 4 * N,
                        ap=[[0, 1], [1, 2 * N]],
                    ),
                )
                bc3s[(bb, hh)] = (t, ejr)

            issue_bc(0, 0)
            issue_bc(0, 1)

            graph_state = {}   # b -> (notm, gext, aj_s, ej_s, fj_s, eif8, out_b)

            def get_graph(b):
                if b not in graph_state:
                    graph_state[b] = (*preps.pop(b), ob_p.tile([128, C, H * D], F32, tag="ob", name=f"ob_{b}"))
                return graph_state[b]

            if True:

                def postproc(b, h, UT_ps):
                    out_b = get_graph(b)[6]
                    # relu fused into the PSUM->SBUF copy: den (row 16) > 0
                    # always, so relu(num)/den == relu(num/den)
                    UT_s = ut_p.tile([D + 1, N], F32)
                    nc.scalar.activation(out=UT_s[:], in_=UT_ps[:], func=AF.Relu)

                    # transpose back to (i, 17) and normalize
                    V_ps = xps_p.tile([128, C, D + 1], F32, tag="xv")
                    for c in range(C):
                        nc.tensor.transpose(
                            V_ps[:, c, :],
                            UT_s[:, c * 128 : (c + 1) * 128],
                            id17_s[:],
                        )
                    rc_s = rc_p.tile([128, C], F32)
                    nc.vector.reciprocal(rc_s[:], V_ps[:, :, D])
                    rc_b = bass.AP(
                        tensor=rc_s.tensor,
                        offset=rc_s.offset,
                        ap=[rc_s.ap[0], rc_s.ap[-1], [0, D]],
                    )
                    nc.vector.tensor_tensor(
                        out=out_b[:, :, h * D : (h + 1) * D],
                        in0=V_ps[:, :, 0:D],
                        in1=rc_b,
                        op=ALU.mult,
                    )

                def phaseA(b, h):
                    notm, gext, aj_s, ej_s, fj_s, eif8, _ = get_graph(b)
                    bc3, ejr = bc3s.pop((b, h))
                    Ei_bc = bc3[:, 0, :]
                    Fi_bc = bc3[:, 1, :]
                    ai_bc = bc3[:, 2, :]

                    def col(t, c):
                        return bass.AP(
                            tensor=t.tensor,
                            offset=t.offset + c * H + h,
                            ap=[t.ap[0], [1, 1]],
                        )

                    groups = []
                    # producer stages for every group (stage-major so no
                    # engine sits waiting on a cross-engine round trip)
                    for gi, g0 in enumerate(range(0, C, GRP)):
                        cs = list(range(g0, g0 + GRP))
                        flavor = PATTERN[((b * H + h) * NG + gi) % len(PATTERN)]
                        if flavor == "OLD":
                            t2 = sct2_p.tile([128, GRP, N], F32, tag="t2", name=f"t2_{b}_{h}_{g0}")
                            for i, c in enumerate(cs):
                                nc.scalar.activation(
                                    out=t2[:, i, :],
                                    in_=ai_bc,
                                    func=AF.Prelu,
                                    bias=col(aj_s, c),
                                    scale=1.0,
                                    alpha=NEG_SLOPE,
                                )
                            w2 = scu_p.tile([128, GRP, N], BF16, tag="u", name=f"ex_{b}_{h}_{g0}")
                            nc.scalar.activation(out=w2[:], in_=t2[:], func=AF.Exp)
                        elif flavor == "PP":
                            # PE builds u = Ej (x) Ei straight into PSUM;
                            # Pool fuses the F-branch + max while reading it
                            w2 = scu_p.tile([128, GRP, N], BF16, tag="u", name=f"u_{b}_{h}_{g0}")
                            for i, c in enumerate(cs):
                                for half in range(2):
                                    sl = slice(half * 512, (half + 1) * 512)
                                    u_ps = up_p.tile([128, 512], F32, tag="up")
                                    nc.tensor.matmul(
                                        u_ps[:],
                                        lhsT=ejr[0:1, 0, c * 128 : (c + 1) * 128],
                                        rhs=ejr[0:1, 1, sl],
                                        start=True,
                                        stop=True,
                                    )
                                    nc.gpsimd.scalar_tensor_tensor(
                                        out=w2[:, i, sl],
                                        in0=Fi_bc[:, sl],
                                        scalar=col(fj_s, c),
                                        in1=u_ps[:],
                                        op0=ALU.mult,
                                        op1=ALU.max,
                                    )
                        else:
                            w2 = scu_p.tile([128, GRP, N], BF16, tag="u", name=f"u_{b}_{h}_{g0}")
                            for i, c in enumerate(cs):
                                nc.vector.tensor_scalar(
                                    out=w2[:, i, :],
                                    in0=Ei_bc,
                                    scalar1=col(ej_s, c),
                                    scalar2=None,
                                    op0=ALU.mult,
                                )
                            if flavor == "NP":
                                for i, c in enumerate(cs):
                                    nc.gpsimd.scalar_tensor_tensor(
                                        out=w2[:, i, :],
                                        in0=Fi_bc,
                                        scalar=col(fj_s, c),
                                        in1=w2[:, i, :],
                                        op0=ALU.mult,
                                        op1=ALU.max,
                                    )
                            else:  # ND
                                v2 = scv_p.tile([128, GRP, N], BF16, tag="v", name=f"v_{b}_{h}_{g0}")
                                for i, c in enumerate(cs):
                                    nc.vector.tensor_scalar(
                                        out=v2[:, i, :],
                                        in0=Fi_bc,
                                        scalar1=col(fj_s, c),
                                        scalar2=None,
                                        op0=ALU.mult,
                                    )
                                nc.vector.tensor_tensor(
                                    out=w2[:], in0=w2[:], in1=v2[:], op=ALU.max
                                )
                        groups.append((g0, cs, w2))
                    return groups

                def phaseB(b, h, groups):
                    notm, gext = get_graph(b)[0], get_graph(b)[1]
                    UT_ps = ups_p.tile([D + 1, N], F32)
                    gh = gext[:, :, h * (D + 1) : (h + 1) * (D + 1)]
                    for g0, cs, w2 in groups:
                        pm = scpm_p.tile([128, GRP, N], BF16, tag="pm", name=f"pm_{b}_{h}_{g0}")
                        nc.vector.tensor_tensor(
                            out=pm[:], in0=w2[:], in1=notm[:, g0 : g0 + GRP, :],
                            op=ALU.mult,
                        )
                        for i, c in enumerate(cs):
                            for half in range(2):
                                sl = slice(half * 512, (half + 1) * 512)
                                nc.tensor.matmul(
                                    UT_ps[:, sl],
                                    lhsT=gh[:, c, :],
                                    rhs=pm[:, i, sl],
                                    start=(c == 0),
                                    stop=(c == C - 1),
                                )
                    return UT_ps

                # flat software pipeline over all (b, h):
                #   phaseA(k) | phaseB(k-1) | postproc(k-2)
                seq = [(b, h) for b in range(B_SH) for h in range(H)]
                stA = {}
                stB = {}
                for k in range(len(seq) + 2):
                    if k < len(seq):
                        b, h = seq[k]
                        stA[k] = phaseA(b, h)
                        if k + 2 < len(seq) and seq[k + 2][0] == b:
                            issue_bc(*seq[k + 2])
                        if h == 2 and b + 1 < B_SH:
                            preps[b + 1] = _prep(b + 1)
                            issue_bc(b + 1, 0)
                            issue_bc(b + 1, 1)
                    if 0 <= k - 1 < len(seq):
                        b1, h1 = seq[k - 1]
                        stB[k - 1] = phaseB(b1, h1, stA.pop(k - 1))
                    if 0 <= k - 2 < len(seq):
                        b2, h2 = seq[k - 2]
                        postproc(b2, h2, stB.pop(k - 2))
                        if h2 == H - 1:
                            out_b = graph_state.pop(b2)[6]
                            nc.sync.dma_start(
                                out=out[b2].rearrange("(c p) d -> p c d", p=128),
                                in_=out_b[:],
                            )

    _wrap_to_json(nc)
    return nc


_NC_CACHE = None


def kernel(h, W, Wal, War, mask):
    global _NC_CACHE
    from concourse.bass_utils import run_bass_kernel_spmd

    h = np.asarray(h, dtype=np.float32)
    W = np.asarray(W, dtype=np.float32)
    Wal = np.asarray(Wal, dtype=np.float32)
    War = np.asarray(War, dtype=np.float32)
    import ml_dtypes

    notm_b16 = (~np.asarray(mask, dtype=bool)).astype(ml_dtypes.bfloat16)

    # Fold weights: wcat = [per-head (W_h | 0)] + [W@Wal] + [W@War]
    wcat = np.zeros((NI, WCOLS), dtype=np.float32)
    for hh in range(H):
        wcat[:, hh * (D + 1) : hh * (D + 1) + D] = W[hh]
        wcat[:, GEXT + hh] = W[hh] @ Wal[hh, :, 0]
        wcat[:, GEXT + H + hh] = W[hh] @ War[hh, :, 0]

    hT = np.ascontiguousarray(h.transpose(0, 2, 1))            # (B, I, N)
    notmT = np.ascontiguousarray(notm_b16.transpose(0, 2, 1))  # (B, j, i)
    id17 = np.eye(D + 1, dtype=np.float32)

    if _NC_CACHE is None:
        _NC_CACHE = build_nc()
    nc = _NC_CACHE

    in_maps = []
    for core in range(NCORES):
        sl = slice(core * B_SH, (core + 1) * B_SH)
        in_maps.append(
            {
                "hT": np.ascontiguousarray(hT[sl]),
                "notmT": np.ascontiguousarray(notmT[sl]),
                "wcat": wcat,
                "id17": id17,
            }
        )

    res = run_bass_kernel_spmd(nc, in_maps, list(range(NCORES)))
    out = np.concatenate([res.results[i]["out"] for i in range(NCORES)], axis=0)
    return out.astype(np.float32)


# revision 24
# speedup vs baseline: 1.0238x; 1.0238x over previous
"""GAT multi-head attention (nn_GATMHAEfficient) on 8 Trainium2 NeuronCores.

Data-parallel over batch B=32 -> 4 graphs per core. Host folds W/Wal/War
into wcat (128 x 152): per-head [W_h | 0] blocks (the 0 column becomes the
on-chip "ones" column so the aggregation matmul also produces the softmax
denominator), then W@Wal (a_i) and W@War (a_j) columns.

Score pipeline: exp(leaky_relu(a_i + a_j)) == max(E_i*E_j, F_i*F_j) with
E = exp(a), F = exp(0.2*a) (exact identity: for s>0 exp(s) wins, for s<0
exp(0.2s) wins). The exps move to the small per-node vectors, so the N^2
work is cheap bf16 ALU ops instead of ACT table ops:
  u  = (Ei_bc * Ej_ptr)            DVE tensor_scalar (bf16 4x mode)
  w  = (Fi_bc * Fj_ptr) max u      Pool STT (fused) or DVE TS+TT
  P  = w * notm                    DVE tensor_tensor (bf16 2x mode)
A tunable fraction of tiles instead uses the direct ACT pipeline
(Prelu(ai_bc + aj_bias) in f32, Exp -> bf16) to load-balance ACT vs
DVE vs Pool. P (bf16) streams into the PE aggregation matmul at
1 cycle/row; per-head [g|1] stationary gives numerator + denominator.
"""

import json

import numpy as np

import concourse.bass as bass
import concourse.mybir as mybir
import concourse.tile as tile
from concourse.vector_clock import ScopedClock, VectorClock

F32 = mybir.dt.float32
BF16 = mybir.dt.bfloat16
AF = mybir.ActivationFunctionType
ALU = mybir.AluOpType

B, N, NI, H, D = 32, 1024, 128, 8, 16
NCORES = 8
B_SH = B // NCORES          # graphs per core
C = N // 128                # j-chunks of 128
NEG_SLOPE = 0.2
GEXT = H * (D + 1)          # 136: per-head [g(16) | ones]
WCOLS = GEXT + 2 * H        # 152: + a_i cols + a_j cols
GRP = 2                     # chunks per score tile
NG = C // GRP               # score tile groups per (b, h)

# flavor pattern over groups: OLD = ACT prelu+exp, NP = DVE-TS + Pool-STT,
# ND = all-DVE. Tuned so ACT/DVE/Pool busy times balance
# (target ~12 OLD / 19 NP / 1 ND per 32).
PATTERN = ["OLD", "NP", "NP", "OLD", "NP", "NP", "OLD", "NP",
           "NP", "OLD", "NP", "NP", "OLD", "NP", "NP", "ND",
           "OLD", "NP", "NP", "OLD", "NP", "NP", "OLD", "NP",
           "NP", "OLD", "NP", "OLD", "NP", "OLD", "NP", "OLD"]

# ---------------------------------------------------------------------------
# Workarounds for this container's walrus build: it accepts at most ONE
# sync-wait per instruction, but Tile's sem-assignment (and its final drain)
# attach several. Split the excess onto dedicated single-wait EventSemaphore
# carrier instructions in the serialized BIR.


def _legalize_sync_waits(d, max_waits=1):
    for fn in d["functions"]:
        for bb in fn["blocks"]:
            new_insts = []
            for inst in bb["instructions"]:
                si = inst.get("sync_info") or {}
                w = si.get("on_wait") or []
                if len(w) > max_waits:
                    for k, we in enumerate(w[:-max_waits]):
                        new_insts.append(
                            {
                                "debug": inst.get("debug", 0),
                                "engine": inst["engine"],
                                "ins": [],
                                "outs": [],
                                "name": f"{inst['name']}_xw{k}",
                                "opcode": "EventSemaphore",
                                "sync_info": {"on_update": [], "on_wait": [we]},
                            }
                        )
                    si["on_wait"] = w[-max_waits:]
                new_insts.append(inst)
            bb["instructions"] = new_insts


def _wrap_to_json(nc):
    raw = nc.to_json_bytes

    def patched():
        d = json.loads(raw())
        _legalize_sync_waits(d)
        return json.dumps(d).encode()

    nc.to_json_bytes = patched


def _split_drain_and_barrier(self, tick_clock, wait_clock):
    # One drain per logical processor so each carries a single sem wait.
    gc = tick_clock.global_clock
    n = len(gc)
    for proc in range(n):
        t = gc[proc]
        if t > 0:
            dr = self.nc.sync.drain()
            pc = VectorClock([t if i == proc else 0 for i in range(n)])
            wait_clock.add_sem_waits(dr.ins, ScopedClock({None: pc}))
    self.nc.all_engine_barrier()
    popped = self.nc._tile_sem_poison_stack.pop()
    assert popped is self._sem_poison
    self.nc.clear_and_free_semaphores(list(self.sems.allocated().values()))
    self.nc.all_engine_barrier()


tile.TileContext._drain_and_barrier = _split_drain_and_barrier

# ---------------------------------------------------------------------------


def build_nc():
    nc = bass.Bass()
    hT = nc.dram_tensor("hT", [B_SH, NI, N], F32, kind="ExternalInput")
    notmT = nc.dram_tensor("notmT", [B_SH, N, N], BF16, kind="ExternalInput")
    wcat = nc.dram_tensor("wcat", [NI, WCOLS], F32, kind="ExternalInput")
    id17 = nc.dram_tensor("id17", [D + 1, D + 1], F32, kind="ExternalInput")
    out = nc.dram_tensor("out", [B_SH, N, H * D], F32, kind="ExternalOutput")
    # per (b,h): rows [Ej | Ei | Fi | ai] staged for the broadcast DMA
    ef_scr = nc.dram_tensor("ef_scr", [B_SH, H, 4, N], BF16)

    from contextlib import ExitStack

    with ExitStack() as ctx:
        tc = ctx.enter_context(tile.TileContext(nc))
        const_p = ctx.enter_context(tc.tile_pool(name="const", bufs=1))
        hb_p = ctx.enter_context(tc.tile_pool(name="hb", bufs=2))
        nm_p = ctx.enter_context(tc.tile_pool(name="nm", bufs=2))
        gx_p = ctx.enter_context(tc.tile_pool(name="gx", bufs=2))
        aj_p = ctx.enter_context(tc.tile_pool(name="aj", bufs=2))
        ei_p = ctx.enter_context(tc.tile_pool(name="ei", bufs=2))
        bc_p = ctx.enter_context(tc.tile_pool(name="bc", bufs=4))
        scu_p = ctx.enter_context(tc.tile_pool(name="scu", bufs=6))
        scv_p = ctx.enter_context(tc.tile_pool(name="scv", bufs=2))
        scpm_p = ctx.enter_context(tc.tile_pool(name="scpm", bufs=3))
        sct2_p = ctx.enter_context(tc.tile_pool(name="sct2", bufs=3))
        ut_p = ctx.enter_context(tc.tile_pool(name="ut", bufs=2))
        rc_p = ctx.enter_context(tc.tile_pool(name="rc", bufs=2))
        ob_p = ctx.enter_context(tc.tile_pool(name="ob", bufs=2))
        xps_p = ctx.enter_context(tc.tile_pool(name="xps", bufs=2, space="PSUM"))
        ups_p = ctx.enter_context(tc.tile_pool(name="ups", bufs=2, space="PSUM"))
        if True:
            wcat_s = const_p.tile([NI, WCOLS], F32)
            nc.sync.dma_start(out=wcat_s[:], in_=wcat[:])
            id17_s = const_p.tile([D + 1, D + 1], F32)
            nc.sync.dma_start(out=id17_s[:], in_=id17[:])
            onec = const_p.tile([128, 1], BF16)
            nc.vector.memset(onec[:], 1.0)

            def _prep(b):
                hbT = hb_p.tile([NI, N], F32)
                nc.sync.dma_start(out=hbT[:], in_=hT[b])

                # a_i rows first (they gate the ef_scr round-trip and the
                # per-head broadcast DMAs), then the X-proj chunks.
                # Half-tiles keep each PSUM tile at one bank.
                # staging rows (1=Ei 2=Fi 3=ai; row 0 unused)
                eif8 = ei_p.tile([H, 4, N], BF16, tag="eif8")
                for half in range(2):
                    sl = slice(half * 512, (half + 1) * 512)
                    XT_ps = xps_p.tile([H, 512], F32, tag="xv")
                    nc.tensor.matmul(
                        XT_ps[:],
                        lhsT=wcat_s[:, GEXT : GEXT + H],
                        rhs=hbT[:, sl],
                        start=True,
                        stop=True,
                    )
                    nc.scalar.activation(out=eif8[:, 1, sl], in_=XT_ps[:], func=AF.Exp, scale=1.0)
                    nc.scalar.activation(out=eif8[:, 2, sl], in_=XT_ps[:], func=AF.Exp, scale=0.2)
                    nc.vector.tensor_copy(eif8[:, 3, sl], XT_ps[:])
                for k in range(1, 4):
                    dst = bass.AP(
                        tensor=ef_scr,
                        offset=(b * H * 4 + k) * N,
                        ap=[[4 * N, H], [1, N]],
                    )
                    nc.sync.dma_start(out=dst, in_=eif8[:, k, :])

                # X = h_b @ wcat chunk by chunk; g -> bf16, a_j cols -> f32
                gext = gx_p.tile([128, C, GEXT], BF16, tag="gx")
                aj_s = aj_p.tile([128, C, H], F32, tag="aj")
                ej_s = aj_p.tile([128, C, H], F32, tag="ej")
                fj_s = aj_p.tile([128, C, H], F32, tag="fj")
                for c in range(C):
                    X_ps = xps_p.tile([128, WCOLS], F32, tag="xv")
                    nc.tensor.matmul(
                        X_ps[:],
                        lhsT=hbT[:, c * 128 : (c + 1) * 128],
                        rhs=wcat_s[:],
                        start=True,
                        stop=True,
                    )
                    if c % 2 == 0:
                        nc.vector.tensor_copy(gext[:, c, :], X_ps[:, 0:GEXT])
                    else:
                        nc.scalar.copy(out=gext[:, c, :], in_=X_ps[:, 0:GEXT])
                    nc.vector.tensor_copy(aj_s[:, c, :], X_ps[:, GEXT + H : WCOLS])
                # per-head ones column (bf16 1.0 exact)
                ones_view = bass.AP(
                    tensor=gext.tensor,
                    offset=gext.offset + D,
                    ap=[gext.ap[0], [GEXT, C], [D + 1, H]],
                )
                ones_src = bass.AP(
                    tensor=onec.tensor,
                    offset=onec.offset,
                    ap=[onec.ap[0], [0, C], [0, H]],
                )
                nc.vector.tensor_copy(ones_view, ones_src)
                # E_j = exp(a_j), F_j = exp(0.2 a_j) per-partition scalars
                nc.scalar.activation(out=ej_s[:], in_=aj_s[:], func=AF.Exp, scale=1.0)
                nc.scalar.activation(out=fj_s[:], in_=aj_s[:], func=AF.Exp, scale=0.2)

                # notm last: it is a big transfer only needed by phase B, and
                # the serial DMA resource must first serve the ef_scr/bc3
                # chain that gates phase A
                notm = nm_p.tile([128, C, N], BF16)
                nc.sync.dma_start(
                    out=notm[:], in_=notmT[b].rearrange("(c p) i -> p c i", p=128)
                )

                return notm, gext, aj_s, ej_s, fj_s, eif8

            preps = {0: _prep(0)}
            bc3s = {}

            def issue_bc(bb, hh):
                t = bc_p.tile([128, 3, N], BF16, tag="bc3", name=f"bc3_{bb}_{hh}")
                nc.sync.dma_start(
                    out=t[:],
                    in_=bass.AP(
                        tensor=ef_scr,
                        offset=(bb * H + hh) * 4 * N + N,
                        ap=[[0, 128], [1, 3 * N]],
                    ),
                )
                bc3s[(bb, hh)] = t

            issue_bc(0, 0)
            issue_bc(0, 1)

            graph_state = {}   # b -> (notm, gext, aj_s, ej_s, fj_s, eif8, out_b)

            def get_graph(b):
                if b not in graph_state:
                    graph_state[b] = (*preps.pop(b), ob_p.tile([128, C, H * D], F32, tag="ob", name=f"ob_{b}"))
                return graph_state[b]

            if True:

                def postproc(b, h, UT_ps):
                    out_b = get_graph(b)[6]
                    # relu fused into the PSUM->SBUF copy: den (row 16) > 0
                    # always, so relu(num)/den == relu(num/den)
                    UT_s = ut_p.tile([D + 1, N], F32)
                    nc.scalar.activation(out=UT_s[:], in_=UT_ps[:], func=AF.Relu)

                    # transpose back to (i, 17) and normalize
                    V_ps = xps_p.tile([128, C, D + 1], F32, tag="xv")
                    for c in range(C):
                        nc.tensor.transpose(
                            V_ps[:, c, :],
                            UT_s[:, c * 128 : (c + 1) * 128],
                            id17_s[:],
                        )
                    rc_s = rc_p.tile([128, C], F32)
                    nc.vector.reciprocal(rc_s[:], V_ps[:, :, D])
                    rc_b = bass.AP(
                        tensor=rc_s.tensor,
                        offset=rc_s.offset,
                        ap=[rc_s.ap[0], rc_s.ap[-1], [0, D]],
                    )
                    nc.vector.tensor_tensor(
                        out=out_b[:, :, h * D : (h + 1) * D],
                        in0=V_ps[:, :, 0:D],
                        in1=rc_b,
                        op=ALU.mult,
                    )

                def phaseA(b, h):
                    notm, gext, aj_s, ej_s, fj_s, eif8, _ = get_graph(b)
                    bc3 = bc3s.pop((b, h))
                    Ei_bc = bc3[:, 0, :]
                    Fi_bc = bc3[:, 1, :]
                    ai_bc = bc3[:, 2, :]

                    def col(t, c):
                        return bass.AP(
                            tensor=t.tensor,
                            offset=t.offset + c * H + h,
                            ap=[t.ap[0], [1, 1]],
                        )

                    groups = []
                    # producer stages for every group (stage-major so no
                    # engine sits waiting on a cross-engine round trip)
                    for gi, g0 in enumerate(range(0, C, GRP)):
                        cs = list(range(g0, g0 + GRP))
                        flavor = PATTERN[((b * H + h) * NG + gi) % len(PATTERN)]
                        if flavor == "OLD":
                            t2 = sct2_p.tile([128, GRP, N], F32, tag="t2", name=f"t2_{b}_{h}_{g0}")
                            for i, c in enumerate(cs):
                                nc.scalar.activation(
                                    out=t2[:, i, :],
                                    in_=ai_bc,
                                    func=AF.Prelu,
                                    bias=col(aj_s, c),
                                    scale=1.0,
                                    alpha=NEG_SLOPE,
                                )
                            w2 = scu_p.tile([128, GRP, N], BF16, tag="u", name=f"ex_{b}_{h}_{g0}")
                            nc.scalar.activation(out=w2[:], in_=t2[:], func=AF.Exp)
                        else:
                            w2 = scu_p.tile([128, GRP, N], BF16, tag="u", name=f"u_{b}_{h}_{g0}")
                            for i, c in enumerate(cs):
                                nc.vector.tensor_scalar(
                                    out=w2[:, i, :],
                                    in0=Ei_bc,
                                    scalar1=col(ej_s, c),
                                    scalar2=None,
                                    op0=ALU.mult,
                                )
                            if flavor == "NP":
                                for i, c in enumerate(cs):
                                    nc.gpsimd.scalar_tensor_tensor(
                                        out=w2[:, i, :],
                                        in0=Fi_bc,
                                        scalar=col(fj_s, c),
                                        in1=w2[:, i, :],
                                        op0=ALU.mult,
                                        op1=ALU.max,
                                    )
                            else:  # ND
                                v2 = scv_p.tile([128, GRP, N], BF16, tag="v", name=f"v_{b}_{h}_{g0}")
                                for i, c in enumerate(cs):
                                    nc.vector.tensor_scalar(
                                        out=v2[:, i, :],
                                        in0=Fi_bc,
                                        scalar1=col(fj_s, c),
                                        scalar2=None,
                                        op0=ALU.mult,
                                    )
                                nc.vector.tensor_tensor(
                                    out=w2[:], in0=w2[:], in1=v2[:], op=ALU.max
                                )
                        groups.append((g0, cs, w2))
                    return groups

                def phaseB(b, h, groups):
                    notm, gext = get_graph(b)[0], get_graph(b)[1]
                    UT_ps = ups_p.tile([D + 1, N], F32)
                    gh = gext[:, :, h * (D + 1) : (h + 1) * (D + 1)]
                    for g0, cs, w2 in groups:
                        pm = scpm_p.tile([128, GRP, N], BF16, tag="pm", name=f"pm_{b}_{h}_{g0}")
                        nc.vector.tensor_tensor(
                            out=pm[:], in0=w2[:], in1=notm[:, g0 : g0 + GRP, :],
                            op=ALU.mult,
                        )
                        for i, c in enumerate(cs):
                            for half in range(2):
                                sl = slice(half * 512, (half + 1) * 512)
                                nc.tensor.matmul(
                                    UT_ps[:, sl],
                                    lhsT=gh[:, c, :],
                                    rhs=pm[:, i, sl],
                                    start=(c == 0),
                                    stop=(c == C - 1),
                                )
                    return UT_ps

                # flat software pipeline over all (b, h):
                #   phaseA(k) | phaseB(k-1) | postproc(k-2)
                seq = [(b, h) for b in range(B_SH) for h in range(H)]
                stA = {}
                stB = {}
                for k in range(len(seq) + 2):
                    if k < len(seq):
                        b, h = seq[k]
                        stA[k] = phaseA(b, h)
                        if k + 2 < len(seq) and seq[k + 2][0] == b:
                            issue_bc(*seq[k + 2])
                        if h == 2 and b + 1 < B_SH:
                            preps[b + 1] = _prep(b + 1)
                            issue_bc(b + 1, 0)
                            issue_bc(b + 1, 1)
                    if 0 <= k - 1 < len(seq):
                        b1, h1 = seq[k - 1]
                        stB[k - 1] = phaseB(b1, h1, stA.pop(k - 1))
                    if 0 <= k - 2 < len(seq):
                        b2, h2 = seq[k - 2]
                        postproc(b2, h2, stB.pop(k - 2))
                        if h2 == H - 1:
                            out_b = graph_state.pop(b2)[6]
                            nc.sync.dma_start(
                                out=out[b2].rearrange("(c p) d -> p c d", p=128),
                                in_=out_b[:],
                            )

    _wrap_to_json(nc)
    return nc


_NC_CACHE = None


def kernel(h, W, Wal, War, mask):
    global _NC_CACHE
    from concourse.bass_utils import run_bass_kernel_spmd

    h = np.asarray(h, dtype=np.float32)
    W = np.asarray(W, dtype=np.float32)
    Wal = np.asarray(Wal, dtype=np.float32)
    War = np.asarray(War, dtype=np.float32)
    import ml_dtypes

    notm_b16 = (~np.asarray(mask, dtype=bool)).astype(ml_dtypes.bfloat16)

    # Fold weights: wcat = [per-head (W_h | 0)] + [W@Wal] + [W@War]
    wcat = np.zeros((NI, WCOLS), dtype=np.float32)
    for hh in range(H):
        wcat[:, hh * (D + 1) : hh * (D + 1) + D] = W[hh]
        wcat[:, GEXT + hh] = W[hh] @ Wal[hh, :, 0]
        wcat[:, GEXT + H + hh] = W[hh] @ War[hh, :, 0]

    hT = np.ascontiguousarray(h.transpose(0, 2, 1))            # (B, I, N)
    notmT = np.ascontiguousarray(notm_b16.transpose(0, 2, 1))  # (B, j, i)
    id17 = np.eye(D + 1, dtype=np.float32)

    if _NC_CACHE is None:
        _NC_CACHE = build_nc()
    nc = _NC_CACHE

    in_maps = []
    for core in range(NCORES):
        sl = slice(core * B_SH, (core + 1) * B_SH)
        in_maps.append(
            {
                "hT": np.ascontiguousarray(hT[sl]),
                "notmT": np.ascontiguousarray(notmT[sl]),
                "wcat": wcat,
                "id17": id17,
            }
        )

    res = run_bass_kernel_spmd(nc, in_maps, list(range(NCORES)))
    out = np.concatenate([res.results[i]["out"] for i in range(NCORES)], axis=0)
    return out.astype(np.float32)
